# revision 49
# baseline (speedup 1.0000x reference)
"""Trainium2 Bass kernel for nn_AttentionModel (patch-transformer + MSE loss).

Model (per batch element b of B=32):
    x[b] : [L=32768] --instance-norm--> patches [T=1024, PS=32]
    h    = patches @ W_proj + b_proj                  [T, 256]
    qkv  = h @ W_qkv + b_qkv ;  q,k,v = split(qkv)    [T, 256] each
    attn = softmax(causal(q k^T / 16))                [T, T]
    out  = (attn @ v) @ W_out + b_out                 [T, 256]
    pred = out @ W_head + b_head                      [T, PS]
    loss = mean((pred[:, :-1] - patches[:, 1:])**2)   scalar

Sharding: data-parallel over batch, 4 batch elements per core x 8 cores.
Each core computes a partial sum-of-squares; host combines.

Key algebraic restructure (host-precomputed constants):
    Xa        = [patches^T ; ones]                [33, T] (normalized on-chip)
    M_qk      = Wq_eff Wk_eff^T                   [33, 33]
    M_vo_aug  = [Wv_eff (W_out W_head), e_ones]   [33, 33]
  where Wq_eff = [W_proj W_qkv_q ; b_q_eff] etc. Then:
    scores^T  = Xa^T M_qk Xa      computed as Xa-dot-(M_qk^T Xa), already
                in [s, t] layout, so no PE transposes of the attention
                probabilities are needed.
    VW_aug    = Xa^T M_vo_aug                     [T, 33]
    predu/css = VW_aug^T exp(scores^T/16)         [33, T]
                rows 0..31 = unnormalized pred^T; row 32 = softmax
                denominator (the e_ones column of M_vo_aug sums exp
                columns through the ones row of Xa).
    pred^T    = predu * (1/css) + b_oh
All attention matmuls are bf16 (errors average out in the final scalar
mean); instance-norm, the softmax exp, denominators, and the loss are fp32.
softmax skips the max-subtraction pass: scores/16 ~ N(0,1) and bounded by
|q||k|/16 < ~40, so exp cannot overflow fp32.
sqrt(var) is computed as exp(0.5*ln(var)) so all ScalarE functions come
from one ACT table set (a Sqrt would force ~2.7us table reloads).
"""

import math
import os

import numpy as np

import concourse.bass as bass
import concourse.mybir as mybir
import concourse.tile as tile
from concourse.bass_utils import run_bass_kernel_spmd
from concourse.masks import make_identity, make_upper_triangular
from concourse.vector_clock import ScopedClock

F32 = mybir.dt.float32
BF16 = mybir.dt.bfloat16
AX = mybir.AxisListType
ALU = mybir.AluOpType
AF = mybir.ActivationFunctionType

N_CORES = 8
B = 32
L = 32768
PS = 32
D = 256
T = L // PS  # 1024
BPC = B // N_CORES  # batch elements per core = 4
NT = T // 128  # 8 s-tiles
KA = PS + 1  # augmented contraction dim (extra ones row)
SCALE = 1.0 / math.sqrt(D)  # 1/16


class SplitDrainTileContext(tile.TileContext):
    """TileContext whose final drain splits sem waits across multiple drain
    instructions -- this walrus rejects >1 sync wait per instruction."""

    def _drain_and_barrier(self, tick_clock, wait_clock):
        probe = mybir.InstDrain(name=f"I-{self.nc.next_id()}", ins=[], outs=[])
        probe.engine = mybir.EngineType.SP
        wait_clock.add_sem_waits(probe, ScopedClock({None: tick_clock.global_clock}))
        waits = list(probe.sync_info.on_wait) if probe.sync_info else []
        assert self.sems is not None
        handles = {h.num: h for h in self.sems.allocated().values()}
        if not waits:
            self.nc.sync.drain()
        for w in waits:
            d = self.nc.sync.drain()
            d.wait_op(handles[w.id], w.wait_value, "sem-ge", check=False)
        self.nc.all_engine_barrier()
        popped = self.nc._tile_sem_poison_stack.pop()
        assert popped is self._sem_poison
        self.nc.clear_and_free_semaphores(list(self.sems.allocated().values()))
        self.nc.all_engine_barrier()


def split_excess_waits(nc, max_waits=1):
    """This walrus rejects instructions carrying more than one sync wait.
    Hoist extra waits onto the immediately preceding same-engine
    instruction when that instruction signals nothing (then waiting before
    it cannot starve anyone), else insert a wait-only drain."""
    for f in nc.m.functions:
        for blk in f.blocks:
            insts = list(blk.instructions)
            out = []
            prev_by_engine = {}
            changed = False
            for inst in insts:
                si = inst.sync_info
                waits = list(si.on_wait) if si else []
                if len(waits) > max_waits:
                    changed = True
                    extra, keep = waits[:-max_waits], waits[-max_waits:]
                    remaining = []
                    prev = prev_by_engine.get(str(inst.engine))
                    for w in extra:
                        psi = prev.sync_info if prev is not None else None
                        if prev is not None and (
                            psi is None
                            or (len(psi.on_wait) == 0 and len(psi.on_update) == 0)
                        ):
                            prev.sync_info = mybir.SyncInfo(on_wait=[w], on_update=[])
                            prev = None  # one hoist per predecessor
                        else:
                            remaining.append(w)
                    for w in remaining:
                        dr = mybir.InstDrain(name=f"I-{nc.next_id()}", ins=[], outs=[])
                        dr.engine = inst.engine
                        dr.sync_info = mybir.SyncInfo(on_wait=[w], on_update=[])
                        out.append(dr)
                    inst.sync_info = mybir.SyncInfo(
                        on_wait=keep, on_update=list(si.on_update)
                    )
                out.append(inst)
                prev_by_engine[str(inst.engine)] = inst
            if changed:
                blk.instructions = out


def dedupe_ldweights(nc):
    """Drop an InstLdweights whose operand AP is byte-identical to the
    immediately preceding PE instruction's InstLdweights (no other PE
    instruction between them) -- the stationary operand is still loaded.
    Only legal when the elided load carries no sync actions."""
    import json as _json

    for f in nc.m.functions:
        for blk in f.blocks:
            insts = list(blk.instructions)
            out = []
            last_pe_ldw_key = None
            changed = False
            for inst in insts:
                if str(inst.engine) != "EngineType.PE":
                    out.append(inst)
                    continue
                tname = type(inst).__name__
                if tname == "InstLdweights":
                    si = inst.sync_info
                    has_sync = si and (len(si.on_wait) or len(si.on_update))
                    try:
                        key = str(inst.ins[0])
                    except Exception:
                        key = None
                    if (
                        key is not None
                        and key == last_pe_ldw_key
                        and not has_sync
                    ):
                        changed = True
                        continue  # elide duplicate load
                    last_pe_ldw_key = key
                    out.append(inst)
                else:
                    if tname == "InstMatmult":
                        # transpose-mode matmuls reload the array themselves
                        if getattr(inst, "is_transpose", None):
                            last_pe_ldw_key = None
                    else:
                        last_pe_ldw_key = None
                    out.append(inst)
            if changed:
                blk.instructions = out


def build_program():
    nc = bass.Bass("TRN2", target_bir_lowering=False, debug=False, num_devices=N_CORES)

    x_d = nc.dram_tensor("x", [BPC, L], F32, kind="ExternalInput")
    mqk_d = nc.dram_tensor("m_qk", [KA, KA], BF16, kind="ExternalInput")
    mvo_d = nc.dram_tensor("m_vo", [KA, KA], BF16, kind="ExternalInput")
    out_d = nc.dram_tensor("loss_partial", [1, 1], F32, kind="ExternalOutput")

    from contextlib import ExitStack

    with SplitDrainTileContext(nc) as tc, ExitStack() as ctx:
        cpool = ctx.enter_context(tc.tile_pool(name="consts", bufs=1))
        ppool_s = ctx.enter_context(tc.tile_pool(name="psum_s", bufs=4, space="PSUM"))
        ppool_o = ctx.enter_context(tc.tile_pool(name="psum_o", bufs=2, space="PSUM"))
        ppool_t = ctx.enter_context(tc.tile_pool(name="psum_t", bufs=2, space="PSUM"))
        xpool = ctx.enter_context(tc.tile_pool(name="xc", bufs=4))
        spool = ctx.enter_context(tc.tile_pool(name="small", bufs=6))
        bigpool = ctx.enter_context(tc.tile_pool(name="big", bufs=5))
        epool = ctx.enter_context(tc.tile_pool(name="et", bufs=2))
        scratch = ctx.enter_context(tc.tile_pool(name="scratch", bufs=4))

        # ---- constants ----
        ident_f = cpool.tile([128, 128], F32)
        make_identity(nc, ident_f[:])
        triu_b = cpool.tile([128, 128], BF16)
        make_upper_triangular(nc, triu_b[:], val=1.0, diag=True)
        ident_b = cpool.tile([128, 128], BF16)
        make_identity(nc, ident_b[:])
        ones_col = cpool.tile([128, 1], F32)
        nc.vector.memset(ones_col[:], 1.0)
        ones_row = cpool.tile([1, PS], F32)
        nc.vector.memset(ones_row[:], 1.0)
        ones_row_b = cpool.tile([1, PS], BF16)
        nc.vector.memset(ones_row_b[:], 1.0)

        mqk = cpool.tile([KA, KA], BF16)
        nc.gpsimd.dma_start(mqk[:], mqk_d.ap()[:])
        mvo = cpool.tile([KA, KA], BF16)
        nc.gpsimd.dma_start(mvo[:], mvo_d.ap()[:])

        lp_all = cpool.tile([PS, BPC], F32)  # per-batch loss partials

        # PE warm-up: the HAM clock gate holds the PE at 1.2 GHz until it
        # sees ~3.4us of sustained activity. The prologue (x DMA + stats)
        # leaves the PE idle, so the first real matmuls all run at half
        # clock. Burn ~3.5us of dummy matmuls during the prologue instead.
        warm_ps = ppool_t.tile([128, 128], F32, tag="pt")
        for _ in range(30):
            nc.tensor.matmul(
                warm_ps[:], triu_b[:], triu_b[:], start=True, stop=True
            )

        for b in range(BPC):
            # ---- A: load x[b] contiguously as [128, 256] ----
            xc = xpool.tile([128, L // 128], F32)
            # partition u, free (k, ps) <- x[b, (128k + u)*32 + ps]: each
            # partition gets 8 tokens at stride 4KB; transposing column block
            # k then yields tokens [128k, 128k+128) contiguously.
            nc.sync.dma_start(
                xc[:].rearrange("u (k ps) -> u k ps", ps=PS),
                x_d.ap()[b].rearrange("(k u ps) -> u k ps", u=128, ps=PS),
            )

            # ---- B: instance-norm stats ----
            sums = spool.tile([128, 2], F32)
            nc.vector.reduce_sum(sums[:, 0:1], xc[:], axis=AX.X)
            sq_scr = scratch.tile([128, L // 128], F32)
            nc.vector.tensor_tensor(out=sq_scr[:], in0=xc[:], in1=xc[:], op=ALU.mult)
            nc.vector.reduce_sum(sums[:, 1:2], sq_scr[:], axis=AX.X)
            tot_ps = ppool_s.tile([1, 2], F32, tag="s")
            nc.tensor.matmul(tot_ps[:], ones_col[:], sums[:], start=True, stop=True)
            tot = spool.tile([1, 2], F32)
            nc.vector.tensor_copy(tot[:], tot_ps[:])

            # sc = [mean, s*m, ssq-s*m, ln, std, std+eps, rstd, -m*rstd, -mean]
            sc = spool.tile([1, 9], F32)
            nc.scalar.mul(sc[:, 0:1], tot[:, 0:1], 1.0 / L)  # mean
            nc.vector.tensor_tensor(
                out=sc[:, 1:2], in0=tot[:, 0:1], in1=sc[:, 0:1], op=ALU.mult
            )
            nc.vector.tensor_tensor(
                out=sc[:, 2:3], in0=tot[:, 1:2], in1=sc[:, 1:2], op=ALU.subtract
            )
            nc.scalar.activation(sc[:, 3:4], sc[:, 2:3], AF.Ln, scale=1.0 / (L - 1))
            nc.scalar.activation(sc[:, 4:5], sc[:, 3:4], AF.Exp, scale=0.5)  # std
            nc.vector.tensor_scalar_add(sc[:, 5:6], sc[:, 4:5], 1e-5)
            nc.vector.reciprocal(sc[:, 6:7], sc[:, 5:6])  # rstd
            nc.scalar.mul(sc[:, 8:9], sc[:, 0:1], -1.0)  # -mean
            nc.vector.tensor_tensor(
                out=sc[:, 7:8], in0=sc[:, 8:9], in1=sc[:, 6:7], op=ALU.mult
            )  # -mean*rstd

            # broadcast [rstd, -mean*rstd] to 32 partitions via rank-1 matmul
            bc_ps = ppool_s.tile([PS, 2], F32, tag="s")
            nc.tensor.matmul(bc_ps[:], ones_row[:], sc[:, 6:8], start=True, stop=True)
            bc = spool.tile([PS, 2], F32)
            nc.vector.tensor_copy(bc[:], bc_ps[:])

            # ---- C: transpose x into patch-major Xa [33, 1024], normalized;
            # row 32 is the constant-1 augmentation row ----
            xnt_b = bigpool.tile([KA, T], BF16, tag="xntb")
            nc.gpsimd.memset(xnt_b[PS : PS + 1, :], 1.0)
            for r in range(2):
                xt_ps = ppool_t.tile([PS, 512], F32, tag="pt")
                for c in range(4):
                    k = 4 * r + c
                    nc.tensor.transpose(
                        xt_ps[:, c * 128 : (c + 1) * 128],
                        xc[:, k * PS : (k + 1) * PS],
                        ident_f[:],
                    )
                # xt_ps[ps, c*128+u] = token 128*(4r+c)+u elem ps: dense write
                nc.vector.tensor_scalar(
                    out=xnt_b[0:PS, r * 512 : (r + 1) * 512],
                    in0=xt_ps[:],
                    scalar1=bc[:, 0:1],
                    scalar2=bc[:, 1:2],
                    op0=ALU.mult,
                    op1=ALU.add,
                )
            # ---- D: Y = M_qk^T Xa  [33, 1024] bf16 ----
            y_b = bigpool.tile([KA, T], BF16, tag="y")
            for n in range(2):
                y_ps = ppool_s.tile([KA, 512], F32, tag="s")
                nc.tensor.matmul(
                    y_ps[:],
                    mqk[:],
                    xnt_b[:, n * 512 : (n + 1) * 512],
                    start=True,
                    stop=True,
                )
                nc.vector.tensor_copy(y_b[:, n * 512 : (n + 1) * 512], y_ps[:])

            # ---- E: VW_aug = Xa^T M_vo_aug  [8][128, 33] bf16; all eight
            # fit one PSUM bank (264 f32), evacuated with a single copy ----
            vw_all = bigpool.tile([128, NT * KA], BF16, tag="vw")
            vw_ps = ppool_s.tile([128, NT * KA], F32, tag="s")
            for j in range(NT):
                nc.tensor.matmul(
                    vw_ps[:, j * KA : (j + 1) * KA],
                    xnt_b[:, j * 128 : (j + 1) * 128],
                    mvo[:],
                    start=True,
                    stop=True,
                )
            nc.vector.tensor_copy(vw_all[:], vw_ps[:])

            # ---- F+G interleaved per t-half: produce the eT chunks a
            # t-half needs, run its PV/pred accumulation, and overlap its
            # normalization epilogue with the next half's eT production ----
            # one eT tile per batch (slices per s-tile j): fewer pool
            # alloc/release sync pairs than eight separate tiles
            et_all = epool.tile([128, NT * T], BF16, tag="et", name=f"et_{b}")
            et = {j: et_all[:, j * T : (j + 1) * T] for j in range(NT)}
            predt = bigpool.tile([PS, T], F32, tag="pred")
            for n in range(2):
                for j in range(4 * n + 4):
                    e_j = et[j]
                    c = n  # chunk index == t-half
                    c0 = (j * 128) // 512
                    if c < c0:
                        continue  # entirely non-causal for this half
                    off = max(0, j * 128 - c * 512)  # within-chunk start
                    sT_ps = ppool_s.tile([128, 512], F32, tag="s")
                    nc.tensor.matmul(
                        sT_ps[:, off:512],
                        xnt_b[:, j * 128 : (j + 1) * 128],
                        y_b[:, c * 512 + off : (c + 1) * 512],
                        start=True,
                        stop=True,
                    )
                    nc.scalar.activation(
                        e_j[:, c * 512 + off : (c + 1) * 512],
                        sT_ps[:, off:512],
                        AF.Exp,
                        scale=SCALE,
                    )
                    if c == c0:
                        # diagonal block: zero the s > t half (keep s <= t)
                        nc.vector.tensor_tensor(
                            out=e_j[:, j * 128 : (j + 1) * 128],
                            in0=e_j[:, j * 128 : (j + 1) * 128],
                            in1=triu_b[:],
                            op=ALU.mult,
                        )
                pu_ps = ppool_o.tile([KA, 512], F32, tag="o")
                for j in range(4 * n + 4):
                    off = max(0, j * 128 - n * 512)
                    nc.tensor.matmul(
                        pu_ps[:, off:512],
                        vw_all[:, j * KA : (j + 1) * KA],
                        et[j][:, n * 512 + off : (n + 1) * 512],
                        start=(j == 0),
                        stop=(j == 4 * n + 3),
                    )
                # pred = pred_u / colsum (+ b_oh via M_vo bias row);
                # 1/colsum = exp(-ln(colsum)) on ScalarE -- DVE reciprocal is
                # an iterative divide (~4.3 cyc/elem) and was the top DVE cost
                lncs = spool.tile([1, 512], F32, tag="rrow")
                nc.scalar.activation(lncs[:], pu_ps[PS : PS + 1, :], AF.Ln)
                rr_b = spool.tile([1, 512], BF16, tag="rrowb")
                nc.scalar.activation(rr_b[:], lncs[:], AF.Exp, scale=-1.0)
                bcr_ps = ppool_t.tile([PS, 512], F32, tag="pt")
                nc.tensor.matmul(
                    bcr_ps[:], ones_row_b[:], rr_b[:], start=True, stop=True
                )
                bcr_sb = scratch.tile([PS, 512], F32, tag="pn")
                nc.vector.tensor_copy(bcr_sb[:], bcr_ps[:])
                nc.vector.tensor_tensor(
                    out=predt[:, n * 512 : (n + 1) * 512],
                    in0=pu_ps[0:PS, :],
                    in1=bcr_sb[:],
                    op=ALU.mult,
                )

            # ---- H: loss partial: sum((pred[:, :-1] - patches[:, 1:])^2) ----
            dd = scratch.tile([PS, T], F32, tag="dd")
            nc.vector.tensor_tensor(
                out=dd[:, 0 : T - 1],
                in0=predt[:, 0 : T - 1],
                in1=xnt_b[0:PS, 1:T],
                op=ALU.subtract,
            )
            nc.scalar.activation(
                dd[:, 0 : T - 1],
                dd[:, 0 : T - 1],
                AF.Square,
                accum_out=lp_all[:, b : b + 1],
            )

        # ---- final: total partial over batches & partitions ----
        lsum = spool.tile([PS, 1], F32)
        nc.vector.reduce_sum(lsum[:], lp_all[:], axis=AX.X)
        tot_ps2 = ppool_s.tile([1, 1], F32, tag="s")
        nc.tensor.matmul(tot_ps2[:], ones_col[0:PS, :], lsum[:], start=True, stop=True)
        out_sb = spool.tile([1, 1], F32)
        nc.vector.tensor_copy(out_sb[:], tot_ps2[:])
        nc.gpsimd.dma_start(out_d.ap()[:], out_sb[:])

    split_excess_waits(nc)
    dedupe_ldweights(nc)
    return nc


_program_cache = {}


def _get_program():
    if "nc" not in _program_cache:
        _program_cache["nc"] = build_program()
    return _program_cache["nc"]


def make_in_maps(x, W_proj, b_proj, W_qkv, b_qkv, W_out, b_out, W_head, b_head):
    import ml_dtypes

    f8 = np.float64
    w_eff = W_proj.astype(f8) @ W_qkv.astype(f8)  # [32, 768]
    b_eff = b_proj.astype(f8) @ W_qkv.astype(f8) + b_qkv.astype(f8)  # [768]
    w_aug = np.concatenate([w_eff, b_eff[None, :]], axis=0)  # [33, 768]
    wq, wk, wv = w_aug[:, 0:D], w_aug[:, D : 2 * D], w_aug[:, 2 * D : 3 * D]
    m_qk = wq @ wk.T  # [33, 33]
    w_oh = W_out.astype(f8) @ W_head.astype(f8)  # [256, 32]
    b_oh = b_out.astype(f8) @ W_head.astype(f8) + b_head.astype(f8)  # [32]
    m_vo = wv @ w_oh  # [33, 32]
    # folding b_oh into the bias row: pred_u' = sum_s (VW + b_oh) eT, so
    # pred_u'/colsum = pred + b_oh exactly.
    m_vo[PS, :] += b_oh
    e_ones = np.zeros((KA, 1), f8)
    e_ones[PS, 0] = 1.0  # selects Xa's ones row -> colsum output column
    m_vo_aug = np.concatenate([m_vo, e_ones], axis=1)  # [33, 33]

    mqk_b = np.ascontiguousarray(m_qk.astype(ml_dtypes.bfloat16))
    mvo_b = np.ascontiguousarray(m_vo_aug.astype(ml_dtypes.bfloat16))

    in_maps = []
    for core in range(N_CORES):
        xs = np.ascontiguousarray(x[core * BPC : (core + 1) * BPC])
        in_maps.append({"x": xs, "m_qk": mqk_b, "m_vo": mvo_b})
    return in_maps


def kernel(**inputs) -> np.ndarray:
    inputs = {k: np.asarray(v) for k, v in inputs.items()}
    nc = _get_program()
    in_maps = make_in_maps(**inputs)
    res = run_bass_kernel_spmd(nc, in_maps, core_ids=list(range(N_CORES)))
    total = sum(float(res.results[i]["loss_partial"][0, 0]) for i in range(N_CORES))
    loss = total / (B * (T - 1) * PS)
    return np.float32(loss)


if __name__ == "__main__":
    rng = np.random.default_rng(0)
    ins = {
        "x": rng.standard_normal((B, L)).astype(np.float32),
        "W_proj": (rng.standard_normal((PS, D)) / math.sqrt(PS)).astype(np.float32),
        "b_proj": np.zeros(D, np.float32),
        "W_qkv": (rng.standard_normal((D, 3 * D)) / math.sqrt(D)).astype(np.float32),
        "b_qkv": np.zeros(3 * D, np.float32),
        "W_out": (rng.standard_normal((D, D)) / math.sqrt(D)).astype(np.float32),
        "b_out": np.zeros(D, np.float32),
        "W_head": (rng.standard_normal((D, PS)) / math.sqrt(D)).astype(np.float32),
        "b_head": np.zeros(PS, np.float32),
    }
    got = kernel(**ins)
    print("kernel loss:", got)
